# revision 46
# baseline (speedup 1.0000x reference)
"""Trainium2 Bass kernel for nn_CapsuleLinear (k-means 'dot' routing, 3 iters).

Math (per example b):
  priors[o,i,v] = sum_l W[o,i,v,l] * x[b,i,l]
  out0 = mean_i priors
  3x: n = normalize(out); logits[o,i] = sum_v priors*n; probs = softmax_o(logits);
      out[o,v] = sum_i probs*priors
  result = squash(out) + bias

Sharding: data-parallel over batch B=64 across 8 cores (8 examples/core).

Per-core layout (P = 128 partitions = (i_p in 0..15, b in 0..7), p = i_p*8+b):
  priors SBUF fp16 [128, ib=32, v=16, o=64], full i = ib*16 + i_p.
  Produced by PE matmuls (block-diag x lhsT vs W2 rhs) into PSUM; the
  PSUM->SBUF fp16 casts are split between ACT (even ib) and DVE (odd ib).
  out0 = sum_i priors: even ibs via PE ones-matmuls (PSUM accumulation),
  odd ibs via a DVE fp16 pairwise tree folded in with a ones-matmul.
  The 0/1 "ones" matrix (1 where p%8 == m%8) reduces the partition dim
  AND re-broadcasts over all i_p rows.
  Routing iterations: DVE does only the two big fp16 muls (priors*n and
  priors*probs, 4 chunks of FD 8192 each) plus the small softmax ops.
  The v-reduction (logits = sum_v prod) runs on the PE as 16 PSUM-
  accumulating identity-matmuls per chunk (strided rhs), pipelined
  behind the DVE mul chunks; exp/zsum are pipelined per 512-col chunk
  on ACT/DVE. The i-reduction (out = sum_i probs*priors) is the PE
  ones-matmul over the prod chunks, also trailing the DVE muls.
"""

import os

import numpy as np

import concourse.bacc as bacc
import concourse.tile as tile
from concourse import mybir
from concourse.bass_utils import run_bass_kernel_spmd

B, I, O, V, L = 64, 512, 64, 16, 8
NCORES = 8
BL = B // NCORES  # 8 examples per core
IB = I // 16  # 32 blocks of 16 i's
NQ = 4  # ib-chunks per pass
QIB = IB // NQ  # 8 ibs per chunk

f32 = mybir.dt.float32
f16 = mybir.dt.float16

LAST_RESULT = None  # stash of BassKernelResults for test harness


def _build_kernel():
    nc = bacc.Bacc(
        "TRN2",
        target_bir_lowering=False,
        debug=False,
        enable_asserts=False,
        num_devices=NCORES,
    )
    w2_d = nc.dram_tensor("w2", [128, IB, O * V], f16, kind="ExternalInput")
    xdg_d = nc.dram_tensor("xdg", [128, IB, 128], f16, kind="ExternalInput")
    ones_d = nc.dram_tensor("onesd", [128, 128], f16, kind="ExternalInput")
    iden_d = nc.dram_tensor("idend", [128, 128], f16, kind="ExternalInput")
    bias_d = nc.dram_tensor("biasT", [V, O], f32, kind="ExternalInput")
    out_d = nc.dram_tensor("out", [BL, V, O], f32, kind="ExternalOutput")

    with tile.TileContext(nc) as tc:
        _body(nc, tc, w2_d, xdg_d, ones_d, iden_d, bias_d, out_d)
    nc.compile()
    return nc


def _body(nc, tc, w2_d, xdg_d, ones_d, iden_d, bias_d, out_d):
    AL = mybir.AluOpType
    X = mybir.AxisListType.X
    AF = mybir.ActivationFunctionType

    from contextlib import ExitStack

    with ExitStack() as ctx:
        big = ctx.enter_context(tc.tile_pool(name="big", bufs=1))
        wp = ctx.enter_context(tc.tile_pool(name="wp", bufs=2))
        sm = ctx.enter_context(tc.tile_pool(name="sm", bufs=1))
        # PSUM: pool A (phase-1 priors pp + per-iter logits halves, 6 banks
        # so the phase-1 matmul->copy pipeline is 3 deep), pool B (the out
        # state, 2 banks; out_prev is always fully consumed by the ntile
        # ops before out_new's first accumulating matmul, so one buffer
        # suffices).
        ps_a = ctx.enter_context(tc.tile_pool(name="psa", bufs=3, space="PSUM"))
        ps_o = ctx.enter_context(tc.tile_pool(name="pso", bufs=1, space="PSUM"))

        # ---- persistent tiles ----
        priors = big.tile([128, IB, V, O], f16)
        prod = big.tile([128, IB, V, O], f16)
        probs = big.tile([128, IB, O], f16)
        elog = big.tile([128, IB, O], f16)
        zs = big.tile([128, IB], f32)
        ones_t = big.tile([128, 128], f16)
        iden_t = big.tile([128, 128], f16)
        bias_t = big.tile([BL, V, O], f32)

        bias8 = sm.tile([128, 1], f32, tag="b8")

        # ---- phase 1: priors + out0 ----
        # The PE does only the 64 priors matmuls. PSUM->SBUF fp16 casts are
        # split ACT:DVE 20:12 (alternating inside each 8-ib window so the
        # PSUM double-buffer pipeline stays PE-bound). out0 = sum_i priors
        # is a DVE fp16 pairwise tree built per 8-ib window (so it overlaps
        # the copies), folded in at the end by a ones-matmul (which also
        # re-broadcasts over the i_p partition rows).
        out0 = ps_o.tile([128, V, O], f32, tag="out")
        out0f = out0[:].rearrange("p v o -> p (v o)")
        # per-window partial sums: st[k, j] fp16, j in 0..3 pair sums
        # tree scratch aliases prod's first 8 ib slots (prod is unused
        # until the iterations)
        st = prod[:, 0:8].rearrange("p (k j) v o -> p k j v o", j=2)
        pr2 = priors[:].rearrange("p (g two) v o -> p g two v o", two=2)
        dve_ibs = {3, 5, 7}  # ib%8 values copied by the DVE (3 of 8)
        # one batched DMA for all the block-diagonal x tiles, and one
        # 4-ib-chunk DMA per 4 weight tiles (the per-tile dma_start issue
        # cost on the SP queue, ~0.6us each, otherwise gates phase 1)
        # Transfers are split across the sync and (otherwise idle) GPSIMD
        # DMA queues so the weight stream isn't bound by one queue's
        # ~350 GB/s; constants ride the gpsimd queue after the early x
        # tiles.
        xdg_t = big.tile([128, IB, 128], f16)
        w4_first = wp.tile([128, 4, O * V], f16, tag="w", name="w4f")
        nc.sync.dma_start(out=w4_first[:], in_=w2_d[:, 0:4])
        nc.scalar.dma_start(out=xdg_t[:, 0:8], in_=xdg_d[:, 0:8])
        for j in range(1, 4):
            nc.gpsimd.dma_start(
                out=xdg_t[:, 8 * j : 8 * j + 8], in_=xdg_d[:, 8 * j : 8 * j + 8]
            )
        nc.gpsimd.dma_start(out=ones_t[:], in_=ones_d[:])
        nc.gpsimd.dma_start(out=iden_t[:], in_=iden_d[:])
        nc.gpsimd.memset(bias8[:], -8.0)
        nc.gpsimd.dma_start(
            out=bias_t[:], in_=bias_d[:].unsqueeze(0).broadcast_to([BL, V, O])
        )
        for ib in range(IB):
            if ib % 4 == 0:
                if ib == 0:
                    w4 = w4_first
                else:
                    w4 = wp.tile([128, 4, O * V], f16, tag="w")
                    # first chunks fan out over 4 queues to kill the DMA
                    # ramp; later ones alternate sync/gpsimd
                    j = ib // 4
                    eng = {1: nc.scalar, 3: nc.scalar}.get(
                        j, nc.sync if j % 2 == 0 else nc.gpsimd
                    )
                    eng.dma_start(out=w4[:], in_=w2_d[:, ib : ib + 4])
            pp = ps_a.tile([128, O * V], f32, tag="pp")
            for h in range(2):
                sl = slice(h * 512, (h + 1) * 512)
                nc.tensor.matmul(
                    pp[:, sl], xdg_t[:, ib], w4[:, ib % 4, sl], start=True, stop=True
                )
            ppv = pp[:].rearrange("p (o v) -> p v o", o=O)
            if ib % 8 in dve_ibs:
                nc.vector.tensor_copy(out=priors[:, ib], in_=ppv)
            else:
                nc.scalar.copy(out=priors[:, ib], in_=ppv)
            if ib % 2 == 0:
                # even ibs: accumulate out0 on the PE (it has slack in the
                # copy-bound phase-1 pipeline)
                pslc = priors[:, ib].rearrange("p v o -> p (v o)")
                for h in range(2):
                    sl = slice(h * 512, (h + 1) * 512)
                    nc.tensor.matmul(
                        out0f[:, sl],
                        ones_t[:],
                        pslc[:, sl],
                        start=(ib == 0),
                        stop=False,
                        skip_group_check=True,
                    )
            if ib % 8 == 7:
                # odd ibs of this window: one paired DVE add (fp16)
                k = ib // 8
                nc.vector.tensor_add(
                    st[:, k], pr2[:, 4 * k : 4 * k + 2, 1], pr2[:, 4 * k + 2 : 4 * k + 4, 1]
                )
        # st[:, k] holds 2 partial sums per window -> [128, 8, V, O] tree
        st8 = st[:].rearrange("p k j v o -> p (k j) v o")
        nc.vector.tensor_add(st8[:, 0:4], st8[:, 0:4], st8[:, 4:8])
        nc.vector.tensor_add(st8[:, 0:2], st8[:, 0:2], st8[:, 2:4])
        nc.vector.tensor_add(st8[:, 0], st8[:, 0], st8[:, 1])
        stf = st8[:, 0].rearrange("p v o -> p (v o)")
        for h in range(2):
            sl = slice(h * 512, (h + 1) * 512)
            nc.tensor.matmul(
                out0f[:, sl],
                ones_t[:],
                stf[:, sl],
                start=False,
                stop=True,
                skip_group_check=True,
            )

        # ---- phase 2: routing iterations ----
        out_prev = out0
        for t in range(3):
            src_vo = out_prev[:]
            src_ov = out_prev[:].transpose([0, 2, 1])  # [128, O, V] view
            # n = out * rsqrt(sum_v out^2)  (scale of out doesn't matter)
            sq = sm.tile([128, O, V], f32, tag="sq")
            nc.scalar.square(sq[:], src_ov)
            nsq = sm.tile([128, O], f32, tag="nsq")
            nc.vector.tensor_reduce(out=nsq[:], in_=sq[:], axis=X, op=AL.add)
            norm = sm.tile([128, O], f32, tag="norm")
            nc.scalar.sqrt(norm[:], nsq[:])
            rn = sm.tile([128, O], f32, tag="rn")
            nc.vector.reciprocal(rn[:], norm[:])
            ntile = sm.tile([128, V, O], f16, tag="ntile")
            nc.vector.tensor_mul(
                ntile[:], src_vo, rn[:].unsqueeze(1).broadcast_to([128, V, O])
            )

            # logits[p, ib, o] = sum_v priors * n
            # DVE: prod chunks (FD 8192); PE: 16 identity-matmuls per chunk
            # accumulate the v-reduction into PSUM; ACT/DVE: exp + zsum per
            # 512-col chunk, all pipelined.
            # two logits halves, 2 banks each
            lg0 = ps_a.tile([128, 2, QIB * O], f32, tag="pp", name="lg0")
            lg1 = ps_a.tile([128, 2, QIB * O], f32, tag="pp", name="lg1")
            lg = [lg0, lg1]
            rz = sm.tile([128, IB], f32, tag="rz")
            for q in range(NQ):
                s = slice(q * QIB, (q + 1) * QIB)
                nc.vector.tensor_mul(
                    prod[:, s],
                    priors[:, s],
                    ntile[:].unsqueeze(1).broadcast_to([128, QIB, V, O]),
                )
                lgq = lg[q // 2][:, q % 2]  # [128, QIB*O] one bank
                pq = prod[:, s]  # [128, QIB, V, O]
                for v in range(V):
                    nc.tensor.matmul(
                        lgq,
                        iden_t[:],
                        pq[:, :, v],
                        start=(v == 0),
                        stop=(v == V - 1),
                        skip_group_check=True,
                    )
                # softmax pieces for this chunk. elog = exp(logits - 8) in
                # fp16 (|logits| <~ 6, so e^(l-8) <= ~1; the constant shift
                # cancels in the softmax) -> the probs mul runs in the DVE
                # 2x perf mode.
                sf = slice(q * QIB, (q + 1) * QIB)
                lgq3 = lgq.rearrange("p (q o) -> p q o", o=O)
                nc.scalar.activation(
                    out=elog[:, sf], in_=lgq3, func=AF.Exp, bias=bias8[:]
                )
                with nc.allow_low_precision(
                    reason="DVE reduces in fp32 internally; fp16 elog input"
                ):
                    nc.vector.tensor_reduce(
                        out=zs[:, sf], in_=elog[:, sf], axis=X, op=AL.add
                    )
                # per-chunk normalize: gives the scheduler DVE-ready work
                # while the last chunk's exp is still in flight, and lets
                # the first out-mul start as soon as its probs are ready
                nc.vector.reciprocal(rz[:, sf], zs[:, sf])
                nc.vector.tensor_mul(
                    probs[:, sf],
                    elog[:, sf],
                    rz[:, sf].unsqueeze(2).broadcast_to([128, QIB, O]),
                )

            # out_new[p, v, o] = sum_i probs * priors. The last 8-ib chunk is
            # split 4+4 so the trailing PE ones-matmuls finish sooner after
            # the final DVE mul.
            out_new = ps_o.tile([128, V, O], f32, tag="out")
            onf = out_new[:].rearrange("p v o -> p (v o)")
            for c0, clen in ((0, 8), (8, 8), (16, 8), (24, 4), (28, 2), (30, 2)):
                s = slice(c0, c0 + clen)
                nc.vector.tensor_mul(
                    prod[:, s],
                    priors[:, s],
                    probs[:, s].unsqueeze(2).broadcast_to([128, clen, V, O]),
                )
                for j in range(clen):
                    ib = c0 + j
                    pslc = prod[:, ib].rearrange("p v o -> p (v o)")
                    for h in range(2):
                        sl = slice(h * 512, (h + 1) * 512)
                        nc.tensor.matmul(
                            onf[:, sl],
                            ones_t[:],
                            pslc[:, sl],
                            start=(ib == 0),
                            stop=(ib == IB - 1),
                            skip_group_check=True,
                        )
            out_prev = out_new

        # ---- squash + bias on partitions 0..7 (b rows) ----
        sq2 = sm.tile([128, O, V], f32, tag="sq")
        src_ov = out_prev[:].transpose([0, 2, 1])
        nc.scalar.square(sq2[:], src_ov)
        nsq2 = sm.tile([128, O], f32, tag="nsq")
        nc.vector.tensor_reduce(out=nsq2[:], in_=sq2[:], axis=X, op=AL.add)
        norm2 = sm.tile([128, O], f32, tag="norm")
        nc.scalar.sqrt(norm2[:], nsq2[:])
        den = sm.tile([128, O], f32, tag="den")
        nc.vector.tensor_scalar_add(den[:], nsq2[:], 1.0)
        rden = sm.tile([128, O], f32, tag="rden")
        nc.vector.reciprocal(rden[:], den[:])
        scl = sm.tile([128, O], f32, tag="scl")
        nc.vector.tensor_mul(scl[:], norm2[:], rden[:])

        outf = sm.tile([BL, V, O], f32, tag="outf")
        nc.vector.tensor_mul(
            outf[:],
            out_prev[0:BL],
            scl[0:BL].unsqueeze(1).broadcast_to([BL, V, O]),
        )
        nc.vector.tensor_add(outf[:], outf[:], bias_t[:])
        nc.sync.dma_start(out=out_d[:], in_=outf[:])


_NC_CACHE = []


def _get_nc():
    if not _NC_CACHE:
        _NC_CACHE.append(_build_kernel())
    return _NC_CACHE[0]


def kernel(x, weight, bias):
    global LAST_RESULT
    x = np.asarray(x, dtype=np.float32)
    weight = np.asarray(weight, dtype=np.float32)
    bias = np.asarray(bias, dtype=np.float32)

    # W2[(i_sub, l), ib, (o, v)] = W[o, ib*16+i_sub, v, l]  (fp16: same byte
    # cost as bf16 but 4x finer mantissa; values are well within fp16 range).
    # Partition-major so a few big DMAs can load it.
    w2 = np.ascontiguousarray(
        weight.transpose(1, 3, 0, 2)
        .reshape(IB, 16, L, O * V)
        .transpose(1, 2, 0, 3)
        .reshape(128, IB, O * V)
    ).astype(np.float16)
    biasT = np.ascontiguousarray(bias.T)  # [V, O]

    idx = np.arange(128)
    onesd = (idx[:, None] % BL == idx[None, :] % BL).astype(np.float16)
    idend = np.eye(128, dtype=np.float16)

    in_maps = []
    for c in range(NCORES):
        xc = x[c * BL : (c + 1) * BL]  # [BL, I, L]
        xt = np.ascontiguousarray(xc.transpose(1, 2, 0))  # [I, L, BL] = (i, l, b)
        xt4 = xt.reshape(IB, 16, L, BL)
        xdg = np.zeros((IB, 128, 128), dtype=np.float16)
        for s in range(16):
            xdg[:, s * L : (s + 1) * L, s * BL : (s + 1) * BL] = xt4[:, s].astype(
                np.float16
            )
        xdg = np.ascontiguousarray(xdg.transpose(1, 0, 2))  # [128, IB, 128]
        in_maps.append(
            {"w2": w2, "xdg": xdg, "onesd": onesd, "idend": idend, "biasT": biasT}
        )

    nc = _get_nc()
    try:
        res = run_bass_kernel_spmd(nc, in_maps, core_ids=list(range(NCORES)))
    except ModuleNotFoundError:
        # BASS_TRACE was set but this environment lacks the axon NTFF hook
        # module; rerun without tracing.
        os.environ["BASS_NEVER_TRACE"] = "1"
        res = run_bass_kernel_spmd(nc, in_maps, core_ids=list(range(NCORES)))
    LAST_RESULT = res

    outs = []
    for r in res.results:
        o = r["out"]  # [BL, V, O]
        outs.append(np.ascontiguousarray(o.transpose(0, 2, 1)))  # [BL, O, V]
    return np.concatenate(outs, axis=0).astype(np.float32)


if __name__ == "__main__":
    rng = np.random.default_rng(0)
    x = rng.standard_normal((B, I, L), dtype=np.float32)
    w = rng.standard_normal((O, I, V, L), dtype=np.float32) * 0.1
    b = rng.standard_normal((O, V), dtype=np.float32) * 0.1
    out = kernel(x, w, b)
    print("out shape", out.shape, out.dtype)


# revision 49
# speedup vs baseline: 1.0080x; 1.0080x over previous
"""Trainium2 Bass kernel for nn_CapsuleLinear (k-means 'dot' routing, 3 iters).

Math (per example b):
  priors[o,i,v] = sum_l W[o,i,v,l] * x[b,i,l]
  out0 = mean_i priors
  3x: n = normalize(out); logits[o,i] = sum_v priors*n; probs = softmax_o(logits);
      out[o,v] = sum_i probs*priors
  result = squash(out) + bias

Sharding: data-parallel over batch B=64 across 8 cores (8 examples/core).

Per-core layout (P = 128 partitions = (i_p in 0..15, b in 0..7), p = i_p*8+b):
  priors SBUF fp16 [128, ib=32, v=16, o=64], full i = ib*16 + i_p.
  Produced by PE matmuls (block-diag x lhsT vs W2 rhs) into PSUM; the
  PSUM->SBUF fp16 casts are split between ACT (even ib) and DVE (odd ib).
  out0 = sum_i priors: even ibs via PE ones-matmuls (PSUM accumulation),
  odd ibs via a DVE fp16 pairwise tree folded in with a ones-matmul.
  The 0/1 "ones" matrix (1 where p%8 == m%8) reduces the partition dim
  AND re-broadcasts over all i_p rows.
  Routing iterations: DVE does only the two big fp16 muls (priors*n and
  priors*probs, 4 chunks of FD 8192 each) plus the small softmax ops.
  The v-reduction (logits = sum_v prod) runs on the PE as 16 PSUM-
  accumulating identity-matmuls per chunk (strided rhs), pipelined
  behind the DVE mul chunks; exp/zsum are pipelined per 512-col chunk
  on ACT/DVE. The i-reduction (out = sum_i probs*priors) is the PE
  ones-matmul over the prod chunks, also trailing the DVE muls.
"""

import os

import numpy as np

import concourse.bacc as bacc
import concourse.tile as tile
from concourse import mybir
from concourse.bass_utils import run_bass_kernel_spmd

B, I, O, V, L = 64, 512, 64, 16, 8
NCORES = 8
BL = B // NCORES  # 8 examples per core
IB = I // 16  # 32 blocks of 16 i's
NQ = 4  # ib-chunks per pass
QIB = IB // NQ  # 8 ibs per chunk

f32 = mybir.dt.float32
f16 = mybir.dt.float16

LAST_RESULT = None  # stash of BassKernelResults for test harness


def _build_kernel():
    nc = bacc.Bacc(
        "TRN2",
        target_bir_lowering=False,
        debug=False,
        enable_asserts=False,
        num_devices=NCORES,
    )
    w2_d = nc.dram_tensor("w2", [128, IB, O * V], f16, kind="ExternalInput")
    xdg_d = nc.dram_tensor("xdg", [128, IB, 128], f16, kind="ExternalInput")
    ones_d = nc.dram_tensor("onesd", [128, 128], f16, kind="ExternalInput")
    iden_d = nc.dram_tensor("idend", [128, 128], f16, kind="ExternalInput")
    bias_d = nc.dram_tensor("biasT", [V, O], f32, kind="ExternalInput")
    out_d = nc.dram_tensor("out", [BL, V, O], f32, kind="ExternalOutput")

    with tile.TileContext(nc) as tc:
        _body(nc, tc, w2_d, xdg_d, ones_d, iden_d, bias_d, out_d)
    nc.compile()
    return nc


def _body(nc, tc, w2_d, xdg_d, ones_d, iden_d, bias_d, out_d):
    AL = mybir.AluOpType
    X = mybir.AxisListType.X
    AF = mybir.ActivationFunctionType

    from contextlib import ExitStack

    with ExitStack() as ctx:
        big = ctx.enter_context(tc.tile_pool(name="big", bufs=1))
        wp = ctx.enter_context(tc.tile_pool(name="wp", bufs=2))
        sm = ctx.enter_context(tc.tile_pool(name="sm", bufs=1))
        # PSUM: pool A (phase-1 priors pp + per-iter logits halves, 6 banks
        # so the phase-1 matmul->copy pipeline is 3 deep), pool B (the out
        # state, 2 banks; out_prev is always fully consumed by the ntile
        # ops before out_new's first accumulating matmul, so one buffer
        # suffices).
        ps_a = ctx.enter_context(tc.tile_pool(name="psa", bufs=3, space="PSUM"))
        ps_o = ctx.enter_context(tc.tile_pool(name="pso", bufs=1, space="PSUM"))

        # ---- persistent tiles ----
        priors = big.tile([128, IB, V, O], f16)
        prod = big.tile([128, IB, V, O], f16)
        probs = big.tile([128, IB, O], f16)
        elog = big.tile([128, IB, O], f16)
        zs = big.tile([128, IB], f32)
        ones_t = big.tile([128, 128], f16)
        iden_t = big.tile([128, 128], f16)
        bias_t = big.tile([BL, V, O], f32)

        bias8 = sm.tile([128, 1], f32, tag="b8")

        # ---- phase 1: priors + out0 ----
        # The PE does only the 64 priors matmuls. PSUM->SBUF fp16 casts are
        # split ACT:DVE 20:12 (alternating inside each 8-ib window so the
        # PSUM double-buffer pipeline stays PE-bound). out0 = sum_i priors
        # is a DVE fp16 pairwise tree built per 8-ib window (so it overlaps
        # the copies), folded in at the end by a ones-matmul (which also
        # re-broadcasts over the i_p partition rows).
        out0 = ps_o.tile([128, V, O], f32, tag="out")
        out0f = out0[:].rearrange("p v o -> p (v o)")
        # per-window partial sums: st[k, j] fp16, j in 0..3 pair sums
        # tree scratch aliases prod's first 8 ib slots (prod is unused
        # until the iterations)
        st = prod[:, 0:8].rearrange("p (k j) v o -> p k j v o", j=2)
        pr2 = priors[:].rearrange("p (g two) v o -> p g two v o", two=2)
        dve_ibs = {3, 5, 7}  # ib%8 values copied by the DVE (3 of 8)
        # one batched DMA for all the block-diagonal x tiles, and one
        # 4-ib-chunk DMA per 4 weight tiles (the per-tile dma_start issue
        # cost on the SP queue, ~0.6us each, otherwise gates phase 1)
        # Transfers are split across the sync and (otherwise idle) GPSIMD
        # DMA queues so the weight stream isn't bound by one queue's
        # ~350 GB/s; constants ride the gpsimd queue after the early x
        # tiles.
        xdg_t = big.tile([128, IB, 128], f16)
        w4_first = wp.tile([128, 4, O * V], f16, tag="w", name="w4f")
        nc.sync.dma_start(out=w4_first[:], in_=w2_d[:, 0:4])
        nc.sync.dma_start(out=xdg_t[:, 0:8], in_=xdg_d[:, 0:8])
        for j in range(1, 4):
            nc.gpsimd.dma_start(
                out=xdg_t[:, 8 * j : 8 * j + 8], in_=xdg_d[:, 8 * j : 8 * j + 8]
            )
        nc.gpsimd.dma_start(out=ones_t[:], in_=ones_d[:])
        nc.gpsimd.dma_start(out=iden_t[:], in_=iden_d[:])
        nc.gpsimd.memset(bias8[:], -8.0)
        nc.gpsimd.dma_start(
            out=bias_t[:], in_=bias_d[:].unsqueeze(0).broadcast_to([BL, V, O])
        )
        for ib in range(IB):
            if ib % 4 == 0:
                if ib == 0:
                    w4 = w4_first
                else:
                    w4 = wp.tile([128, 4, O * V], f16, tag="w")
                    eng = nc.sync if (ib // 4) % 2 == 0 else nc.gpsimd
                    eng.dma_start(out=w4[:], in_=w2_d[:, ib : ib + 4])
            pp = ps_a.tile([128, O * V], f32, tag="pp")
            for h in range(2):
                sl = slice(h * 512, (h + 1) * 512)
                nc.tensor.matmul(
                    pp[:, sl], xdg_t[:, ib], w4[:, ib % 4, sl], start=True, stop=True
                )
            ppv = pp[:].rearrange("p (o v) -> p v o", o=O)
            if ib % 8 in dve_ibs:
                nc.vector.tensor_copy(out=priors[:, ib], in_=ppv)
            else:
                nc.scalar.copy(out=priors[:, ib], in_=ppv)
            if ib % 2 == 0:
                # even ibs: accumulate out0 on the PE (it has slack in the
                # copy-bound phase-1 pipeline)
                pslc = priors[:, ib].rearrange("p v o -> p (v o)")
                for h in range(2):
                    sl = slice(h * 512, (h + 1) * 512)
                    nc.tensor.matmul(
                        out0f[:, sl],
                        ones_t[:],
                        pslc[:, sl],
                        start=(ib == 0),
                        stop=False,
                        skip_group_check=True,
                    )
            if ib % 8 == 7:
                # odd ibs of this window: one paired DVE add (fp16)
                k = ib // 8
                nc.vector.tensor_add(
                    st[:, k], pr2[:, 4 * k : 4 * k + 2, 1], pr2[:, 4 * k + 2 : 4 * k + 4, 1]
                )
        # st[:, k] holds 2 partial sums per window -> [128, 8, V, O] tree
        st8 = st[:].rearrange("p k j v o -> p (k j) v o")
        nc.vector.tensor_add(st8[:, 0:4], st8[:, 0:4], st8[:, 4:8])
        nc.vector.tensor_add(st8[:, 0:2], st8[:, 0:2], st8[:, 2:4])
        nc.vector.tensor_add(st8[:, 0], st8[:, 0], st8[:, 1])
        stf = st8[:, 0].rearrange("p v o -> p (v o)")
        for h in range(2):
            sl = slice(h * 512, (h + 1) * 512)
            nc.tensor.matmul(
                out0f[:, sl],
                ones_t[:],
                stf[:, sl],
                start=False,
                stop=True,
                skip_group_check=True,
            )

        # ---- phase 2: routing iterations ----
        out_prev = out0
        for t in range(3):
            src_vo = out_prev[:]
            src_ov = out_prev[:].transpose([0, 2, 1])  # [128, O, V] view
            # n = out * rsqrt(sum_v out^2)  (scale of out doesn't matter)
            sq = sm.tile([128, O, V], f32, tag="sq")
            nc.scalar.square(sq[:], src_ov)
            nsq = sm.tile([128, O], f32, tag="nsq")
            nc.vector.tensor_reduce(out=nsq[:], in_=sq[:], axis=X, op=AL.add)
            norm = sm.tile([128, O], f32, tag="norm")
            nc.scalar.sqrt(norm[:], nsq[:])
            rn = sm.tile([128, O], f32, tag="rn")
            nc.vector.reciprocal(rn[:], norm[:])
            ntile = sm.tile([128, V, O], f16, tag="ntile")
            nc.vector.tensor_mul(
                ntile[:], src_vo, rn[:].unsqueeze(1).broadcast_to([128, V, O])
            )

            # logits[p, ib, o] = sum_v priors * n
            # DVE: prod chunks (FD 8192); PE: 16 identity-matmuls per chunk
            # accumulate the v-reduction into PSUM; ACT/DVE: exp + zsum per
            # 512-col chunk, all pipelined.
            # two logits halves, 2 banks each
            lg0 = ps_a.tile([128, 2, QIB * O], f32, tag="pp", name="lg0")
            lg1 = ps_a.tile([128, 2, QIB * O], f32, tag="pp", name="lg1")
            lg = [lg0, lg1]
            rz = sm.tile([128, IB], f32, tag="rz")
            for ci, (c0, clen) in enumerate(
                ((0, 8), (8, 8), (16, 8), (24, 4), (28, 4))
            ):
                s = slice(c0, c0 + clen)
                nc.vector.tensor_mul(
                    prod[:, s],
                    priors[:, s],
                    ntile[:].unsqueeze(1).broadcast_to([128, clen, V, O]),
                )
                # chunks 0-2 use full 512-col banks; the tapered last two
                # share lg1's second bank in 256-col halves
                if ci < 2:
                    lgq = lg[0][:, ci]
                elif ci == 2:
                    lgq = lg[1][:, 0]
                else:
                    lgq = lg[1][:, 1][:, (ci - 3) * 256 : (ci - 2) * 256]
                pq = prod[:, s]  # [128, clen, V, O]
                for v in range(V):
                    nc.tensor.matmul(
                        lgq,
                        iden_t[:],
                        pq[:, :, v],
                        start=(v == 0),
                        stop=(v == V - 1),
                        skip_group_check=True,
                    )
                # softmax pieces for this chunk. elog = exp(logits - 8) in
                # fp16 (|logits| <~ 6, so e^(l-8) <= ~1; the constant shift
                # cancels in the softmax) -> the probs mul runs in the DVE
                # 2x perf mode.
                lgq3 = lgq.rearrange("p (q o) -> p q o", o=O)
                nc.scalar.activation(
                    out=elog[:, s], in_=lgq3, func=AF.Exp, bias=bias8[:]
                )
                with nc.allow_low_precision(
                    reason="DVE reduces in fp32 internally; fp16 elog input"
                ):
                    nc.vector.tensor_reduce(
                        out=zs[:, s], in_=elog[:, s], axis=X, op=AL.add
                    )
                # per-chunk normalize: gives the scheduler DVE-ready work
                # while the last chunk's exp is still in flight, and lets
                # the first out-mul start as soon as its probs are ready
                nc.vector.reciprocal(rz[:, s], zs[:, s])
                nc.vector.tensor_mul(
                    probs[:, s],
                    elog[:, s],
                    rz[:, s].unsqueeze(2).broadcast_to([128, clen, O]),
                )

            # out_new[p, v, o] = sum_i probs * priors. The last 8-ib chunk is
            # split 4+4 so the trailing PE ones-matmuls finish sooner after
            # the final DVE mul.
            out_new = ps_o.tile([128, V, O], f32, tag="out")
            onf = out_new[:].rearrange("p v o -> p (v o)")
            for c0, clen in ((0, 8), (8, 8), (16, 8), (24, 4), (28, 2), (30, 2)):
                s = slice(c0, c0 + clen)
                nc.vector.tensor_mul(
                    prod[:, s],
                    priors[:, s],
                    probs[:, s].unsqueeze(2).broadcast_to([128, clen, V, O]),
                )
                for j in range(clen):
                    ib = c0 + j
                    pslc = prod[:, ib].rearrange("p v o -> p (v o)")
                    for h in range(2):
                        sl = slice(h * 512, (h + 1) * 512)
                        nc.tensor.matmul(
                            onf[:, sl],
                            ones_t[:],
                            pslc[:, sl],
                            start=(ib == 0),
                            stop=(ib == IB - 1),
                            skip_group_check=True,
                        )
            out_prev = out_new

        # ---- squash + bias on partitions 0..7 (b rows) ----
        sq2 = sm.tile([128, O, V], f32, tag="sq")
        src_ov = out_prev[:].transpose([0, 2, 1])
        nc.scalar.square(sq2[:], src_ov)
        nsq2 = sm.tile([128, O], f32, tag="nsq")
        nc.vector.tensor_reduce(out=nsq2[:], in_=sq2[:], axis=X, op=AL.add)
        norm2 = sm.tile([128, O], f32, tag="norm")
        nc.scalar.sqrt(norm2[:], nsq2[:])
        den = sm.tile([128, O], f32, tag="den")
        nc.vector.tensor_scalar_add(den[:], nsq2[:], 1.0)
        rden = sm.tile([128, O], f32, tag="rden")
        nc.vector.reciprocal(rden[:], den[:])
        scl = sm.tile([128, O], f32, tag="scl")
        nc.vector.tensor_mul(scl[:], norm2[:], rden[:])

        outf = sm.tile([BL, V, O], f32, tag="outf")
        nc.vector.tensor_mul(
            outf[:],
            out_prev[0:BL],
            scl[0:BL].unsqueeze(1).broadcast_to([BL, V, O]),
        )
        nc.vector.tensor_add(outf[:], outf[:], bias_t[:])
        nc.sync.dma_start(out=out_d[:], in_=outf[:])


_NC_CACHE = []


def _get_nc():
    if not _NC_CACHE:
        _NC_CACHE.append(_build_kernel())
    return _NC_CACHE[0]


def kernel(x, weight, bias):
    global LAST_RESULT
    x = np.asarray(x, dtype=np.float32)
    weight = np.asarray(weight, dtype=np.float32)
    bias = np.asarray(bias, dtype=np.float32)

    # W2[(i_sub, l), ib, (o, v)] = W[o, ib*16+i_sub, v, l]  (fp16: same byte
    # cost as bf16 but 4x finer mantissa; values are well within fp16 range).
    # Partition-major so a few big DMAs can load it.
    w2 = np.ascontiguousarray(
        weight.transpose(1, 3, 0, 2)
        .reshape(IB, 16, L, O * V)
        .transpose(1, 2, 0, 3)
        .reshape(128, IB, O * V)
    ).astype(np.float16)
    biasT = np.ascontiguousarray(bias.T)  # [V, O]

    idx = np.arange(128)
    onesd = (idx[:, None] % BL == idx[None, :] % BL).astype(np.float16)
    idend = np.eye(128, dtype=np.float16)

    in_maps = []
    for c in range(NCORES):
        xc = x[c * BL : (c + 1) * BL]  # [BL, I, L]
        xt = np.ascontiguousarray(xc.transpose(1, 2, 0))  # [I, L, BL] = (i, l, b)
        xt4 = xt.reshape(IB, 16, L, BL)
        xdg = np.zeros((IB, 128, 128), dtype=np.float16)
        for s in range(16):
            xdg[:, s * L : (s + 1) * L, s * BL : (s + 1) * BL] = xt4[:, s].astype(
                np.float16
            )
        xdg = np.ascontiguousarray(xdg.transpose(1, 0, 2))  # [128, IB, 128]
        in_maps.append(
            {"w2": w2, "xdg": xdg, "onesd": onesd, "idend": idend, "biasT": biasT}
        )

    nc = _get_nc()
    try:
        res = run_bass_kernel_spmd(nc, in_maps, core_ids=list(range(NCORES)))
    except ModuleNotFoundError:
        # BASS_TRACE was set but this environment lacks the axon NTFF hook
        # module; rerun without tracing.
        os.environ["BASS_NEVER_TRACE"] = "1"
        res = run_bass_kernel_spmd(nc, in_maps, core_ids=list(range(NCORES)))
    LAST_RESULT = res

    outs = []
    for r in res.results:
        o = r["out"]  # [BL, V, O]
        outs.append(np.ascontiguousarray(o.transpose(0, 2, 1)))  # [BL, O, V]
    return np.concatenate(outs, axis=0).astype(np.float32)


if __name__ == "__main__":
    rng = np.random.default_rng(0)
    x = rng.standard_normal((B, I, L), dtype=np.float32)
    w = rng.standard_normal((O, I, V, L), dtype=np.float32) * 0.1
    b = rng.standard_normal((O, V), dtype=np.float32) * 0.1
    out = kernel(x, w, b)
    print("out shape", out.shape, out.dtype)
